# revision 1
# baseline (speedup 1.0000x reference)
"""HOI relation-scoring kernel for Trainium2 (8 NeuronCores, data-parallel).

Full inputs in, full output out. Internally: batch dim (16 images) is
sharded 2-per-core across 8 cores; MLP weights are replicated.

Per-core pipeline (per image):
  1. Box fields are DMA'd (strided) into a [1, 96] row, cast to f32,
     1/area computed, and broadcast to all 128 partitions via a K=1
     ones-matmul -> bcast [128, 120] = (y1|y2|x1|x2|inv_area) x 24 boxes.
  2. A [128, 43, 24] f32 indicator mask (maskT[yx_chunk_row, chunk, box])
     is built with 7 DVE ops comparing host-constant flat-coordinate
     grids against the broadcast box fields.
  3. ROI sums: features[b] viewed [5476, 768] stream through the PE in
     43 chunks of [128, 768]; mask chunk [128, 24] is the stationary
     operand -> psum [24, 768] accumulates box sums. This makes the
     kernel HBM-bound (the features read is the roofline).
  4. Six PE transposes produce roiT [768(d) x 24(box)], scaled by
     inv_area during psum eviction -> ROI means, transposed layout.
  5. pairs@W1 is factorized: A.T = (hf@W1[:768]).T, B.T = (of@W1[768:]).T
     computed directly in [512, 24] orientation; pair expansion
     h1.T[d1, 16i+j] = A.T[d1, i] + b1 + B.T[d1, j] is a single fused
     scalar_tensor_tensor with broadcast APs; ReLU on ScalarE.
  6. Stages 2/3 chain in transposed layout (h1T as rhs, then h2T as
     lhsT) with no further transposes; biases are per-partition (b2) or
     a DMA-broadcast row tile (b3).
"""

import sys

import numpy as np

for _p in ("/opt/trn_rl_repo",):
    if _p not in sys.path:
        sys.path.insert(0, _p)

from contextlib import ExitStack

from concourse import bacc, mybir, tile
from concourse.bass import ts
from concourse.bass_utils import run_bass_kernel_spmd
from concourse.masks import make_identity

# Problem shapes (hardcoded per contract).
B, H, W, D = 16, 74, 74, 768
NH, NO = 8, 16
NB = NH + NO  # 24 boxes per image
NREL = 117
D1, D2 = 512, 256
NCORES = 8
BPC = B // NCORES  # images per core
YX = H * W  # 5476
NCHUNK = (YX + 127) // 128  # 43
TAIL = YX - 128 * (NCHUNK - 1)  # 100
NPAIR = NH * NO  # 128 pairs per image

F32 = mybir.dt.float32
I32 = mybir.dt.int32

_CACHE = {}


def _coord_consts():
    """Host constants: flat-index -> (y, x) coordinate grids, [128, NCHUNK].

    ycо[p, k] = (128*k + p) // W for valid flat indices, else -1e9 so all
    box compares fail and tail rows contribute zero.
    """
    flat = np.arange(NCHUNK * 128)
    valid = flat < YX
    y = np.where(valid, flat // W, -1e9).astype(np.float32)
    x = np.where(valid, flat % W, -1e9).astype(np.float32)
    yco = np.ascontiguousarray(y.reshape(NCHUNK, 128).T)
    xco = np.ascontiguousarray(x.reshape(NCHUNK, 128).T)
    return yco, xco


def _build_nc(repeat=1):
    import os

    variant = os.environ.get("KBENCH", "full")  # full | nomlp | dmaonly
    nc = bacc.Bacc("TRN2", target_bir_lowering=False)

    feats = nc.dram_tensor("feats", [BPC, H, W, D], F32, kind="ExternalInput")
    hbox = nc.dram_tensor("hbox", [BPC, NH, 4], I32, kind="ExternalInput")
    obox = nc.dram_tensor("obox", [BPC, NO, 4], I32, kind="ExternalInput")
    w1 = nc.dram_tensor("w1", [2 * D, D1], F32, kind="ExternalInput")
    b1 = nc.dram_tensor("b1", [D1], F32, kind="ExternalInput")
    w2 = nc.dram_tensor("w2", [D1, D2], F32, kind="ExternalInput")
    b2 = nc.dram_tensor("b2", [D2], F32, kind="ExternalInput")
    w3 = nc.dram_tensor("w3", [D2, NREL], F32, kind="ExternalInput")
    b3 = nc.dram_tensor("b3", [NREL], F32, kind="ExternalInput")
    yco = nc.dram_tensor("yco", [128, NCHUNK], F32, kind="ExternalInput")
    xco = nc.dram_tensor("xco", [128, NCHUNK], F32, kind="ExternalInput")
    out = nc.dram_tensor("out", [BPC * NPAIR, NREL], F32, kind="ExternalOutput")

    K1 = 2 * D // 128  # 12 chunks of W1 rows (first 6 = human half)
    K2 = D1 // 128  # 4 chunks of W2 rows
    K3 = D2 // 128  # 2 chunks of W3 rows
    MC1 = D1 // 128  # 4 output chunks of stage 1
    MC2 = D2 // 128  # 2 output chunks of stage 2
    DCH = D // 128  # 6 chunks of the feature dim

    with tile.TileContext(nc) as tc, ExitStack() as ctx:
        const = ctx.enter_context(tc.tile_pool(name="const", bufs=1))
        fpool = ctx.enter_context(tc.tile_pool(name="fpool", bufs=8))
        mpool = ctx.enter_context(tc.tile_pool(name="mpool", bufs=2))
        spool = ctx.enter_context(tc.tile_pool(name="spool", bufs=2))
        roi_ps = ctx.enter_context(tc.tile_pool(name="roi_ps", bufs=2, space="PSUM"))
        ppool = ctx.enter_context(tc.tile_pool(name="ppool", bufs=3, space="PSUM"))

        # ---------------- preamble: weights + constants ----------------
        w1sb = []
        w1v = w1[:].rearrange("(c p) m -> c p m", p=128)
        for c in range(K1):
            t = const.tile([128, D1], F32, tag=f"w1_{c}")
            nc.sync.dma_start(t[:], w1v[c])
            w1sb.append(t)
        w2sb = []
        w2v = w2[:].rearrange("(c p) m -> c p m", p=128)
        for c in range(K2):
            t = const.tile([128, D2], F32, tag=f"w2_{c}")
            nc.sync.dma_start(t[:], w2v[c])
            w2sb.append(t)
        w3sb = []
        w3v = w3[:].rearrange("(c p) m -> c p m", p=128)
        for c in range(K3):
            t = const.tile([128, NREL], F32, tag=f"w3_{c}")
            nc.sync.dma_start(t[:], w3v[c])
            w3sb.append(t)

        b1sb = []
        for mc in range(MC1):
            t = const.tile([128, 1], F32, tag=f"b1_{mc}")
            nc.sync.dma_start(t[:], b1[ts(mc, 128)][:, None])
            b1sb.append(t)
        b2sb = []
        for mc in range(MC2):
            t = const.tile([128, 1], F32, tag=f"b2_{mc}")
            nc.sync.dma_start(t[:], b2[ts(mc, 128)][:, None])
            b2sb.append(t)
        b3bc = const.tile([128, NREL], F32, tag="b3bc")
        nc.sync.dma_start(b3bc[:], b3[None, :].to_broadcast((128, NREL)))

        ycosb = const.tile([128, NCHUNK], F32, tag="ycosb")
        nc.sync.dma_start(ycosb[:], yco[:])
        xcosb = const.tile([128, NCHUNK], F32, tag="xcosb")
        nc.sync.dma_start(xcosb[:], xco[:])

        ident = const.tile([128, 128], F32, tag="ident")
        make_identity(nc, ident[:])
        ones_row = const.tile([1, 128], F32, tag="ones_row")
        nc.vector.memset(ones_row[:], 1.0)

        for b in [i % BPC for i in range(BPC * repeat)]:
            fv = feats[b].rearrange("h w d -> (h w) d")  # [5476, 768]

            # ---- box fields -> [1, 120] row: y1 | y2 | x1 | x2 | 1/area
            boxi = spool.tile([1, 4 * NB], I32, tag="boxi")
            for fi, col in enumerate((1, 3, 0, 2)):  # (x1,y1,x2,y2) -> y1,y2,x1,x2
                nc.sync.dma_start(
                    boxi[:, fi * NB : fi * NB + NH], hbox[b, :, col][None, :]
                )
                nc.sync.dma_start(
                    boxi[:, fi * NB + NH : (fi + 1) * NB], obox[b, :, col][None, :]
                )
            boxf = spool.tile([1, 5 * NB], F32, tag="boxf")
            nc.vector.tensor_copy(boxf[:, 0 : 4 * NB], boxi[:])
            dy = spool.tile([1, NB], F32, tag="dy")
            nc.vector.tensor_sub(dy[:], boxf[:, NB : 2 * NB], boxf[:, 0:NB])
            dx = spool.tile([1, NB], F32, tag="dx")
            nc.vector.tensor_sub(dx[:], boxf[:, 3 * NB : 4 * NB], boxf[:, 2 * NB : 3 * NB])
            nc.vector.tensor_mul(dy[:], dy[:], dx[:])
            nc.vector.reciprocal(boxf[:, 4 * NB : 5 * NB], dy[:])

            # ---- broadcast the 120 fields to all partitions (K=1 matmul)
            bps = ppool.tile([128, 5 * NB], F32, tag="pp")
            nc.tensor.matmul(bps[:], ones_row[:], boxf[:], start=True, stop=True)
            bcast = spool.tile([128, 5 * NB], F32, tag="bcast")
            nc.scalar.copy(bcast[:], bps[:])

            # ---- indicator mask [128, NCHUNK, NB]
            mask = mpool.tile([128, NCHUNK, NB], F32, tag="mask")
            mtmp = mpool.tile([128, NCHUNK, NB], F32, tag="mtmp")
            shp = (128, NCHUNK, NB)
            yv = ycosb[:].unsqueeze(2).to_broadcast(shp)
            xv = xcosb[:].unsqueeze(2).to_broadcast(shp)
            y1v = bcast[:, 0:NB].unsqueeze(1).to_broadcast(shp)
            y2v = bcast[:, NB : 2 * NB].unsqueeze(1).to_broadcast(shp)
            x1v = bcast[:, 2 * NB : 3 * NB].unsqueeze(1).to_broadcast(shp)
            x2v = bcast[:, 3 * NB : 4 * NB].unsqueeze(1).to_broadcast(shp)
            nc.vector.tensor_tensor(mask[:], yv, y1v, mybir.AluOpType.is_ge)
            nc.vector.tensor_tensor(mtmp[:], yv, y2v, mybir.AluOpType.is_lt)
            nc.vector.tensor_mul(mask[:], mask[:], mtmp[:])
            nc.vector.tensor_tensor(mtmp[:], xv, x1v, mybir.AluOpType.is_ge)
            nc.vector.tensor_mul(mask[:], mask[:], mtmp[:])
            nc.vector.tensor_tensor(mtmp[:], xv, x2v, mybir.AluOpType.is_lt)
            nc.vector.tensor_mul(mask[:], mask[:], mtmp[:])

            # ---- ROI sums: stream features, mask stationary
            pa = roi_ps.tile([NB, 512], F32, tag="roiA")
            pb = roi_ps.tile([NB, 256], F32, tag="roiB")
            for k in range(NCHUNK):
                rows = 128 if k < NCHUNK - 1 else TAIL
                ft = fpool.tile([128, D], F32, tag="feat")
                nc.sync.dma_start(ft[:rows, :], fv[k * 128 : k * 128 + rows, :])
                if variant == "dmaonly":
                    continue
                lhs = mask[:rows, k, :]
                nc.tensor.matmul(
                    pa[:], lhs, ft[:rows, 0:512], start=(k == 0), stop=(k == NCHUNK - 1)
                )
                nc.tensor.matmul(
                    pb[:], lhs, ft[:rows, 512:768], start=(k == 0), stop=(k == NCHUNK - 1)
                )
            if variant == "dmaonly":
                continue
            roi = spool.tile([NB, D], F32, tag="roi")
            nc.vector.tensor_copy(roi[:, 0:512], pa[:])
            nc.vector.tensor_copy(roi[:, 512:768], pb[:])

            # ---- transpose to roiT [128, DCH, NB], fold in 1/area
            roit = spool.tile([128, DCH, NB], F32, tag="roit")
            for t6 in range(DCH):
                pt = ppool.tile([128, NB], F32, tag="pp")
                nc.tensor.transpose(pt[:], roi[:, ts(t6, 128)], ident[:NB, :NB])
                nc.vector.tensor_mul(
                    roit[:, t6, :], pt[:], bcast[:, 4 * NB : 5 * NB]
                )

            # ---- stage 1: A.T | B.T -> pair-expand -> relu -> h1T
            h1sb = []
            for mc in range(MC1):
                p1 = ppool.tile([128, NB], F32, tag="pp")
                for kc in range(DCH):
                    nc.tensor.matmul(
                        p1[:, 0:NH],
                        w1sb[kc][:, ts(mc, 128)],
                        roit[:, kc, 0:NH],
                        start=(kc == 0),
                        stop=(kc == DCH - 1),
                    )
                for kc in range(DCH):
                    nc.tensor.matmul(
                        p1[:, NH:NB],
                        w1sb[DCH + kc][:, ts(mc, 128)],
                        roit[:, kc, NH:NB],
                        start=(kc == 0),
                        stop=(kc == DCH - 1),
                    )
                ab = spool.tile([128, NB], F32, tag="ab")
                nc.scalar.copy(ab[:], p1[:])
                pre = spool.tile([128, NH, NO], F32, tag="pre")
                nc.vector.scalar_tensor_tensor(
                    pre[:],
                    ab[:, 0:NH].unsqueeze(2).to_broadcast((128, NH, NO)),
                    b1sb[mc][:],
                    ab[:, NH:NB].unsqueeze(1).to_broadcast((128, NH, NO)),
                    mybir.AluOpType.add,
                    mybir.AluOpType.add,
                )
                h1 = spool.tile([128, NPAIR], F32, tag=f"h1_{mc}")
                nc.scalar.activation(h1[:], pre[:], mybir.ActivationFunctionType.Relu)
                h1sb.append(h1)

            # ---- stage 2: h2T[m2] = relu(W2[:, m2].T @ h1 + b2)
            h2sb = []
            for m2 in range(MC2):
                p2 = ppool.tile([128, NPAIR], F32, tag="pp")
                for kc in range(K2):
                    nc.tensor.matmul(
                        p2[:],
                        w2sb[kc][:, ts(m2, 128)],
                        h1sb[kc][:],
                        start=(kc == 0),
                        stop=(kc == K2 - 1),
                    )
                h2 = spool.tile([128, NPAIR], F32, tag=f"h2_{m2}")
                nc.scalar.activation(
                    h2[:], p2[:], mybir.ActivationFunctionType.Relu, bias=b2sb[m2][:]
                )
                h2sb.append(h2)

            # ---- stage 3: out = h2 @ W3 + b3
            p3 = ppool.tile([NPAIR, NREL], F32, tag="pp")
            for kc in range(K3):
                nc.tensor.matmul(
                    p3[:], h2sb[kc][:], w3sb[kc][:], start=(kc == 0), stop=(kc == K3 - 1)
                )
            osb = spool.tile([NPAIR, NREL], F32, tag="osb")
            nc.vector.tensor_add(osb[:], p3[:], b3bc[:])
            nc.sync.dma_start(out[ts(b, NPAIR), :], osb[:])

    nc.compile()
    return nc


def _get_nc(repeat=1):
    key = f"nc{repeat}"
    if key not in _CACHE:
        _CACHE[key] = _build_nc(repeat)
    return _CACHE[key]


def _in_maps(inputs):
    feats = np.ascontiguousarray(np.asarray(inputs["features"], dtype=np.float32))
    hb = np.ascontiguousarray(np.asarray(inputs["human_boxes"], dtype=np.int32))
    ob = np.ascontiguousarray(np.asarray(inputs["obj_boxes"], dtype=np.int32))
    yco, xco = _coord_consts()
    common = {
        "w1": np.ascontiguousarray(np.asarray(inputs["W1"], dtype=np.float32)),
        "b1": np.ascontiguousarray(np.asarray(inputs["b1"], dtype=np.float32)),
        "w2": np.ascontiguousarray(np.asarray(inputs["W2"], dtype=np.float32)),
        "b2": np.ascontiguousarray(np.asarray(inputs["b2"], dtype=np.float32)),
        "w3": np.ascontiguousarray(np.asarray(inputs["W3"], dtype=np.float32)),
        "b3": np.ascontiguousarray(np.asarray(inputs["b3"], dtype=np.float32)),
        "yco": yco,
        "xco": xco,
    }
    maps = []
    for c in range(NCORES):
        m = dict(common)
        m["feats"] = np.ascontiguousarray(feats[c * BPC : (c + 1) * BPC])
        m["hbox"] = np.ascontiguousarray(hb[c * BPC : (c + 1) * BPC])
        m["obox"] = np.ascontiguousarray(ob[c * BPC : (c + 1) * BPC])
        maps.append(m)
    return maps


def run(trace=False, **inputs):
    nc = _get_nc()
    res = run_bass_kernel_spmd(nc, _in_maps(inputs), list(range(NCORES)), trace=trace)
    out = np.concatenate([res.results[c]["out"] for c in range(NCORES)], axis=0)
    return out.astype(np.float32), res


def timed_run(iters=20, repeat=1, **inputs):
    """Mirror bass2jax.run_bass_via_pjrt's 8-core shard_map path, but stage
    inputs on device once and time repeated executions. Returns
    (full_output, best_wall_ns) where best_wall_ns = min over iters of one
    sharded dispatch (upper bound on per-core HW exec time)."""
    import time

    import jax
    from jax.sharding import Mesh, PartitionSpec
    from jax.experimental.shard_map import shard_map

    from concourse import bass2jax, mybir as _mybir

    nc = _get_nc(repeat)
    in_maps = _in_maps(inputs)
    n_cores = NCORES

    partition_name = nc.partition_id_tensor.name if nc.partition_id_tensor else None
    in_names, out_names, out_avals, zero_outs = [], [], [], []
    for alloc in nc.m.functions[0].allocations:
        if not isinstance(alloc, _mybir.MemoryLocationSet):
            continue
        name = alloc.memorylocations[0].name
        if alloc.kind == "ExternalInput":
            if name != partition_name:
                in_names.append(name)
        elif alloc.kind == "ExternalOutput":
            shape = tuple(alloc.tensor_shape)
            dtype = _mybir.dt.np(alloc.dtype)
            out_names.append(name)
            out_avals.append(jax.core.ShapedArray(shape, dtype))
            zero_outs.append(np.zeros(shape, dtype))
    n_params = len(in_names)
    n_outs = len(out_avals)
    all_in_names = list(in_names) + list(out_names)
    if partition_name is not None:
        all_in_names.append(partition_name)
    donate = tuple(range(n_params, n_params + n_outs))

    def _body(*args):
        operands = list(args)
        if partition_name is not None:
            operands.append(bass2jax.partition_id_tensor())
        outs = bass2jax._bass_exec_p.bind(
            *operands,
            out_avals=tuple(out_avals),
            in_names=tuple(all_in_names),
            out_names=tuple(out_names),
            lowering_input_output_aliases=(),
            sim_require_finite=True,
            sim_require_nnan=True,
            nc=nc,
        )
        return tuple(outs)

    bass2jax.install_neuronx_cc_hook()
    devices = jax.devices()[:n_cores]
    mesh = Mesh(np.asarray(devices), ("core",))
    in_specs = (PartitionSpec("core"),) * (n_params + n_outs)
    out_specs = (PartitionSpec("core"),) * len(out_names)
    sharded = jax.jit(
        shard_map(_body, mesh=mesh, in_specs=in_specs, out_specs=out_specs,
                  check_rep=False),
        donate_argnums=donate,
        keep_unused=True,
    )
    per_core = [[np.asarray(m[name]) for name in in_names] for m in in_maps]
    concat_in = [
        np.concatenate([per_core[c][i] for c in range(n_cores)], axis=0)
        for i in range(n_params)
    ]
    concat_zeros = [
        np.zeros((n_cores * z.shape[0], *z.shape[1:]), z.dtype) for z in zero_outs
    ]
    sharding = jax.sharding.NamedSharding(mesh, PartitionSpec("core"))
    dev_in = [jax.device_put(a, sharding) for a in concat_in]
    out_arrs = None
    best = None
    for _ in range(iters):
        dev_zeros = [jax.device_put(z, sharding) for z in concat_zeros]
        jax.block_until_ready(dev_zeros)
        t0 = time.perf_counter()
        res = sharded(*dev_in, *dev_zeros)
        jax.block_until_ready(res)
        dt = time.perf_counter() - t0
        if best is None or dt < best:
            best = dt
            out_arrs = res
    outs = [
        np.asarray(out_arrs[i]).reshape(n_cores, *out_avals[i].shape)
        for i in range(n_outs)
    ]
    full = np.concatenate([outs[out_names.index("out")][c] for c in range(n_cores)], 0)
    return full.astype(np.float32), int(best * 1e9)


def kernel(**inputs):
    out, _ = run(trace=False, **inputs)
    return out

